# revision 19
# baseline (speedup 1.0000x reference)
"""Trainium2 Bass kernel for a 30-expert MLP ensemble.

Computes out[b] = mean_e sigmoid(relu(x @ W1[e] + b1[e]) @ W2[e] + b2[e])
for x [8192, 1024], W1 [30, 1024, 2048], W2 [30, 2048].

Strategy: data-parallel over the batch axis. Each of the 8 cores gets a
1024-row shard of x (pre-transposed on host) and the full replicated
weight stack. No collectives: the host concatenates the 8 disjoint
output shards.

fc1 runs in fp8 (e4m3) with MatmulPerfMode.DoubleRow: two 128-wide
k-subtiles per matmul at 2 moving-rows/cycle, 2x the bf16 PE rate. W1
is scaled by 64 before the fp8 cast so its U(-1/32..) values sit in
e4m3's normal range; the 1/64 folds back in via the Relu eviction
scale. fc1 is 3840 N=512 DoubleRow matmuls at the ~216ns issue floor
(~830us of PE streaming per core) and is the roofline for this kernel.

fc2 runs OFF the PE (as replicated-stationary matmuls it would cost
~106us of PE time for 0.1% of the FLOPs). The Relu eviction writes h
as bf16; the otherwise-idle Vector engine computes
  s[p, b] += h[p, b] * W2[e, jb*128+p]
per column tile as a 4x-mode tensor_scalar multiply (per-partition
scalar) plus a 2x-mode tensor_tensor add. After 16 tiles, one matmul
per (expert, batch-half) with a [128, 1] ones stationary reduces s
over partitions into the logit (~216ns each, 60 total). Sigmoid reads
the 1-partition PSUM row, adds b2, and stages to the bf16 o_all tile;
the expert mean over experts 0..28 runs as a small matmul during
expert 29, and expert 29's contribution is folded in by a DVE
scalar_tensor_tensor (o29/E + mean_psum) so the post-loop tail is just
sigmoid -> DVE combine -> output DMA.

Scheduling: all boot-critical input DMA rides the Sync HWDGE queue in
just-in-time FIFO order (every logical DMA queue shares one ~213GB/s
pool of 16 SDMA engines, so ordering — not queue count — is what
matters; only the tiny expert-0/1 slices of b1/w2 load early, the bulk
follows the xt stream). The PE is warmed out of its cold p-state by
junk N=128 matmuls on a memset tile starting right after the fixed
~7us NEFF preamble. Experts 0 and 29 process all of batch-half 0
before batch-half 1: expert 0 so the PE stream is dense before xt's
second half has landed (a mid-ramp gap would re-arm the HAM throttle
window), expert 29 so batch-half 0's logit/sigmoid/output drains while
batch-half 1 is still in fc1, leaving a minimal tail. w1 is prefetched
8 pair-blocks deep (the split experts hold all 8 of their blocks
across both batch-half passes).

Layouts (prepared host-side in numpy, fp8 = ml_dtypes.float8_e4m3):
  xt  [BH, 128, IB, 512]      xt[t,p,s,n]    = x[t*512+n, s*128 + p]
  w1  [E*JP, 128, 2, IB, 128] w1[gp,p,u,s,q] = 64*W1[e, s*128+p, (2jp+u)*128+q]
  b1  [128, E*JB]             b1[p, e*JB+jb] = b1[e, jb*128 + p]
  w2  [128, E*JB]  (fp32)     w2[p, e*JB+jb] = W2[e, jb*128 + p]
  b2  [1, E]
"""

import numpy as np

import concourse.bacc as bacc
import concourse.mybir as mybir
import concourse.tile as tile
from concourse.bass_utils import run_bass_kernel_spmd

N_CORES = 8
P = 128
NB = 512  # matmul moving free dim (psum bank = 512 fp32)
SCALE = 64.0  # host-side premultiply of W1 before the fp8 cast

E_FULL, I_FULL, H_FULL, B_FULL = 30, 1024, 2048, 8192


def build_bass(E=E_FULL, I=I_FULL, H=H_FULL, BC=B_FULL // N_CORES):
    IB = I // P
    JB = H // P
    JP = JB // 2
    BH = BC // NB
    f32 = mybir.dt.float32
    f8 = mybir.dt.float8e4
    bf16 = mybir.dt.bfloat16
    DoubleRow = mybir.MatmulPerfMode.DoubleRow
    Relu = mybir.ActivationFunctionType.Relu
    Sigmoid = mybir.ActivationFunctionType.Sigmoid
    mult = mybir.AluOpType.mult
    add = mybir.AluOpType.add

    nc = bacc.Bacc(None, target_bir_lowering=False)
    xt_d = nc.dram_tensor("xt", [BC // NB, P, IB, NB], f8, kind="ExternalInput")
    w1_d = nc.dram_tensor("w1", [E * JP, P, 2, IB, P], f8, kind="ExternalInput")
    b1_d = nc.dram_tensor("b1", [P, E * JB], f32, kind="ExternalInput")
    w2_d = nc.dram_tensor("w2", [P, E * JB], f32, kind="ExternalInput")
    b2_d = nc.dram_tensor("b2", [1, E], f32, kind="ExternalInput")
    out_d = nc.dram_tensor("out", [1, BC], f32, kind="ExternalOutput")

    SPLIT = (0, E - 1)  # experts processed one batch-half at a time

    with tile.TileContext(nc) as tc:
        with (
            tc.tile_pool(name="const", bufs=1) as const_pool,
            tc.tile_pool(name="xt", bufs=1) as xt_pool,
            tc.tile_pool(name="w1", bufs=12) as w1_pool,
            tc.tile_pool(name="h", bufs=6) as h_pool,
            tc.tile_pool(name="t", bufs=4) as t_pool,
            tc.tile_pool(name="s", bufs=4) as s_pool,
            tc.tile_pool(name="osb", bufs=3) as osb_pool,
            tc.tile_pool(name="fc1ps", bufs=2, space="PSUM") as fc1_psum,
            tc.tile_pool(name="rps", bufs=2, space="PSUM") as r_psum,
            tc.tile_pool(name="mps", bufs=2, space="PSUM") as m_psum,
        ):
            # --- boot: all latency-critical DMA on the Sync HWDGE queue in
            # just-in-time FIFO order (every logical queue shares the same 16
            # SDMA engines, ~213GB/s total — ordering, not queue count, is
            # what matters); b1/w2/b2 trickle in on GpSimd's SWDGE queue ---
            w1_pre = {}

            def ensure_pre(gpi):
                if gpi < E * JP and gpi not in w1_pre:
                    w_t = w1_pool.tile(
                        [P, 2, IB, P], f8, tag="w1", name=f"w1p_{gpi}"
                    )
                    nc.sync.dma_start(w_t[:], w1_d[gpi])
                    w1_pre[gpi] = w_t

            xt_t = xt_pool.tile([P, BH, IB, NB], f8)
            b1_t = const_pool.tile([P, E * JB], f32)
            w2_t = const_pool.tile([P, E * JB], f32)
            b2_t = const_pool.tile([1, E], f32)
            ensure_pre(0)
            nc.sync.dma_start(xt_t[:, 0, 0:4, :], xt_d[0, :, 0:4, :])
            nc.sync.dma_start(xt_t[:, 0, 4:IB, :], xt_d[0, :, 4:IB, :])
            ensure_pre(1)
            # expert 0/1 bias + fc2-weight columns only (32KB) — the bulk
            # follows after the latency-critical stream
            nc.sync.dma_start(b1_t[:, 0:2 * JB], b1_d[:, 0:2 * JB])
            nc.sync.dma_start(w2_t[:, 0:2 * JB], w2_d[:, 0:2 * JB])
            nc.sync.dma_start(b2_t[:], b2_d[:])
            ensure_pre(2)
            ensure_pre(3)
            nc.sync.dma_start(xt_t[:, 1, 0:4, :], xt_d[1, :, 0:4, :])
            ensure_pre(4)
            nc.sync.dma_start(xt_t[:, 1, 4:IB, :], xt_d[1, :, 4:IB, :])
            for gpi in range(5, JP):
                ensure_pre(gpi)
            nc.sync.dma_start(b1_t[:, 2 * JB:], b1_d[:, 2 * JB:])
            nc.sync.dma_start(w2_t[:, 2 * JB:], w2_d[:, 2 * JB:])

            junk_t = const_pool.tile([P, 2 * P], bf16)
            nc.vector.memset(junk_t[:], 0.03)
            ones_t = const_pool.tile([P, 1], bf16)
            nc.vector.memset(ones_t[:], 1.0)
            dumm_t = const_pool.tile([1, 1], bf16)
            # load the sigmoid table set (contains Relu as filler) now, so no
            # ACT_TABLE_LOAD stalls the eviction stream mid-kernel
            nc.scalar.activation(dumm_t[:], junk_t[0:1, 0:1], Sigmoid)
            wm_ps = fc1_psum.tile([P, 2, NB], f32, tag="fc1", name="warm_ps")
            for wi in range(34):
                nc.tensor.matmul(
                    wm_ps[:, 0, 0:P],
                    junk_t[:, 0:P],
                    junk_t[:, P:2 * P],
                    start=True,
                    stop=True,
                )

            # expert-mean weights for experts 0..E-2; expert E-1 is folded in
            # by the final DVE combine
            cA_t = const_pool.tile([E - 1, 1], bf16)
            nc.any.memset(cA_t[:], 1.0 / E)
            o_all = const_pool.tile([E, BC], bf16)

            s_by_e = {}
            mean_ps = {}

            def dve_fold(e, jb, h_ap, s_ap, shape_tag):
                """s += h * w2[:, col] on the Vector engine (4x mul + 2x add)."""
                col = e * JB + jb
                w2col = w2_t[:, col:col + 1]
                if jb == 0:
                    nc.vector.tensor_scalar_mul(s_ap, h_ap, w2col)
                else:
                    t_t = t_pool.tile(
                        [P, 2, NB], bf16, tag="t", name=f"t_{col}_{shape_tag}"
                    )
                    t_ap = t_t[:, 0:h_ap.shape[1], :] if len(h_ap.shape) == 3 \
                        else t_t[:, 0, :]
                    nc.vector.tensor_scalar_mul(t_ap, h_ap, w2col)
                    nc.vector.tensor_add(s_ap, t_ap, s_ap)

            def fc1_step(e, jp):
                """fc1 for hidden pair (e, jp), both batch halves: 16
                DoubleRow matmuls + 2 Relu evictions + DVE w2-fold."""
                gp = e * JP + jp
                ensure_pre(gp + 8)
                if jp == 0:
                    s_by_e[e] = s_pool.tile(
                        [P, 2, NB], bf16, tag="s", name=f"s_{e}"
                    )
                w_t = w1_pre.pop(gp)
                for u in range(2):
                    jb = 2 * jp + u
                    col = e * JB + jb
                    ps = fc1_psum.tile(
                        [P, 2, NB], f32, tag="fc1", name=f"fc1ps_{col}"
                    )
                    for sb in range(0, IB, 2):
                        for bh in range(BH):
                            nc.tensor.matmul(
                                ps[:, bh, :],
                                w_t[:, u, sb:sb + 2, :],
                                xt_t[:, bh, sb:sb + 2, :],
                                start=(sb == 0),
                                stop=(sb == IB - 2),
                                perf_mode=DoubleRow,
                            )
                    h_t = h_pool.tile(
                        [P, 2, NB], bf16, tag="h", name=f"h_{col}"
                    )
                    nc.scalar.activation(
                        h_t[:],
                        ps[:],
                        Relu,
                        bias=b1_t[:, col:col + 1],
                        scale=1.0 / SCALE,
                    )
                    dve_fold(e, jb, h_t[:], s_by_e[e][:], "f")

            def split_group(e, jp, u, bh, w_t):
                """fc1 for hidden pair (e, jp), u, ONE batch half."""
                jb = 2 * jp + u
                col = e * JB + jb
                ps = fc1_psum.tile(
                    [P, 2, NB], f32, tag="fc1", name=f"fc1ps_{col}_{bh}"
                )
                for sb in range(0, IB, 2):
                    nc.tensor.matmul(
                        ps[:, bh, :],
                        w_t[:, u, sb:sb + 2, :],
                        xt_t[:, bh, sb:sb + 2, :],
                        start=(sb == 0),
                        stop=(sb == IB - 2),
                        perf_mode=DoubleRow,
                    )
                h_t = h_pool.tile(
                    [P, 2, NB], bf16, tag="h", name=f"h_{col}_{bh}"
                )
                nc.scalar.activation(
                    h_t[:, bh, :],
                    ps[:, bh, :],
                    Relu,
                    bias=b1_t[:, col:col + 1],
                    scale=1.0 / SCALE,
                )
                dve_fold(e, jb, h_t[:, bh, :], s_by_e[e][:, bh, :], f"s{bh}")

            def finish_expert(e):
                """partition-reduce s -> logit, sigmoid, stage to o_all."""
                s_t = s_by_e.pop(e)
                for bh in range(BH):
                    r_t = r_psum.tile([1, NB], f32, tag="r", name=f"r_{e}_{bh}")
                    nc.tensor.matmul(
                        r_t[:], ones_t[:], s_t[:, bh, :], start=True, stop=True
                    )
                    o_stage = osb_pool.tile(
                        [1, NB], bf16, tag="ostage", name=f"osig_{e}_{bh}"
                    )
                    nc.scalar.activation(
                        o_stage[:],
                        r_t[:],
                        Sigmoid,
                        bias=b2_t[0:1, e:e + 1],
                        scale=1.0,
                    )
                    nc.sync.dma_start(
                        o_all[e:e + 1, bh * NB:(bh + 1) * NB], o_stage[:]
                    )

            def mean_partial(bh):
                mps = m_psum.tile([1, NB], f32, tag="m", name=f"meanps_{bh}")
                nc.tensor.matmul(
                    mps[:],
                    cA_t[:],
                    o_all[0:E - 1, bh * NB:(bh + 1) * NB],
                    start=True,
                    stop=True,
                )
                mean_ps[bh] = mps

            def finish_last_chunk(bh, lo, hi):
                """expert E-1, batch [bh*NB+lo, bh*NB+hi): reduce, sigmoid,
                fold into the expert mean on DVE, write the output shard."""
                e = E - 1
                w = hi - lo
                r_t = r_psum.tile(
                    [1, NB], f32, tag="r", name=f"r_{e}_{bh}_{lo}"
                )
                nc.tensor.matmul(
                    r_t[0:1, 0:w],
                    ones_t[:],
                    s_by_e[e][:, bh, lo:hi],
                    start=True,
                    stop=True,
                )
                o_stage = osb_pool.tile(
                    [1, NB], bf16, tag="olast", name=f"osig_{e}_{bh}_{lo}"
                )
                nc.scalar.activation(
                    o_stage[0:1, 0:w], r_t[0:1, 0:w], Sigmoid,
                    bias=b2_t[0:1, e:e + 1], scale=1.0,
                )
                o_sb = osb_pool.tile(
                    [1, NB], f32, tag="ofin", name=f"of_{bh}_{lo}"
                )
                nc.vector.scalar_tensor_tensor(
                    o_sb[0:1, 0:w], o_stage[0:1, 0:w], 1.0 / E,
                    mean_ps[bh][0:1, lo:hi], mult, add
                )
                nc.scalar.dma_start(
                    out_d[0:1, bh * NB + lo:bh * NB + hi], o_sb[0:1, 0:w]
                )

            def tail_group(e, jp, u, bh, w_t):
                """like split_group, but in two 256-batch chunks with a
                single-op DVE fold and per-chunk finish, to minimize the
                post-loop dependency chain. Only used for the very last
                column of the last expert."""
                jb = 2 * jp + u
                col = e * JB + jb
                w2col = w2_t[:, col:col + 1]
                ps = fc1_psum.tile(
                    [P, 2, NB], f32, tag="fc1", name=f"fc1ps_{col}_{bh}"
                )
                h_t = h_pool.tile([P, 2, NB], bf16, tag="h", name=f"h_{col}_{bh}")
                HB = NB // 2
                for hf in range(2):
                    lo, hi = hf * HB, (hf + 1) * HB
                    for sb in range(0, IB, 2):
                        nc.tensor.matmul(
                            ps[:, bh, lo:hi],
                            w_t[:, u, sb:sb + 2, :],
                            xt_t[:, bh, sb:sb + 2, lo:hi],
                            start=(sb == 0),
                            stop=(sb == IB - 2),
                            perf_mode=DoubleRow,
                        )
                    nc.scalar.activation(
                        h_t[:, bh, lo:hi],
                        ps[:, bh, lo:hi],
                        Relu,
                        bias=b1_t[:, col:col + 1],
                        scale=1.0 / SCALE,
                    )
                    nc.vector.scalar_tensor_tensor(
                        s_by_e[e][:, bh, lo:hi],
                        h_t[:, bh, lo:hi],
                        w2col,
                        s_by_e[e][:, bh, lo:hi],
                        mult,
                        add,
                    )
                    finish_last_chunk(bh, lo, hi)

            def emit_split(e):
                w1_hold = {}
                for bh in range(BH):
                    for jp in range(JP):
                        gp = e * JP + jp
                        ensure_pre(gp + 8)
                        if bh == 0:
                            if jp == 0:
                                s_by_e[e] = s_pool.tile(
                                    [P, 2, NB], bf16, tag="s", name=f"s_{e}"
                                )
                            w1_hold[jp] = w1_pre.pop(gp)
                        w_t = w1_hold[jp]
                        for u in range(2):
                            if e == E - 1 and bh == 1 and jp == JP - 1 and u == 1:
                                tail_group(e, jp, u, bh, w_t)
                            else:
                                split_group(e, jp, u, bh, w_t)
                        if e == E - 1:
                            if bh == 0 and jp == 0:
                                finish_expert(e - 1)
                            if jp == 4:
                                mean_partial(bh)
                            if bh == 1 and jp == 0:
                                finish_last_chunk(0, 0, NB)
                if e == E - 1:
                    s_by_e.pop(e)
                w1_hold.clear()

            for e in range(E):
                if e in SPLIT:
                    emit_split(e)
                else:
                    for jp in range(JP):
                        fc1_step(e, jp)
                        if jp == 0 and e >= 1:
                            finish_expert(e - 1)
    nc.compile()
    return nc


def prep_inputs(x, W1, b1, W2, b2, E, I, H, BC):
    IB = I // P
    JB = H // P
    JP = JB // 2
    f8 = mybir.dt.np(mybir.dt.float8e4)
    w1_l = np.ascontiguousarray(
        (W1.astype(np.float32) * SCALE)
        .reshape(E, IB, P, JP, 2, P)
        .transpose(0, 3, 2, 4, 1, 5)
        .reshape(E * JP, P, 2, IB, P)
    ).astype(f8)
    b1_l = np.ascontiguousarray(
        b1.reshape(E, JB, P).transpose(2, 0, 1).reshape(P, E * JB), np.float32
    )
    w2_l = np.ascontiguousarray(
        W2.astype(np.float32).reshape(E, JB, P).transpose(2, 0, 1).reshape(P, E * JB)
    )
    b2_l = np.ascontiguousarray(b2.reshape(1, E), np.float32)
    in_maps = []
    for c in range(N_CORES):
        xc = np.asarray(x[c * BC:(c + 1) * BC], np.float32)  # [BC, I]
        xt = np.ascontiguousarray(
            xc.reshape(BC // 512, 512, IB, P).transpose(0, 3, 2, 1)
        ).astype(f8)
        in_maps.append({"xt": xt, "w1": w1_l, "b1": b1_l, "w2": w2_l, "b2": b2_l})
    return in_maps


def run(x, W1, b1, W2, b2, trace=False):
    E, I, H = W1.shape
    BC = x.shape[0] // N_CORES
    in_maps = prep_inputs(x, W1, b1, W2, b2, E, I, H, BC)
    nc = build_bass(E=E, I=I, H=H, BC=BC)
    res = run_bass_kernel_spmd(nc, in_maps, list(range(N_CORES)), trace=trace)
    outs = [res.results[c]["out"].reshape(BC) for c in range(N_CORES)]
    full = np.concatenate(outs)[:, None].astype(np.float32)
    return full, res


def kernel(x, W1, b1, W2, b2):
    out, _ = run(
        np.asarray(x), np.asarray(W1), np.asarray(b1), np.asarray(W2), np.asarray(b2)
    )
    return out


# revision 20
# speedup vs baseline: 1.0017x; 1.0017x over previous
"""Trainium2 Bass kernel for a 30-expert MLP ensemble.

Computes out[b] = mean_e sigmoid(relu(x @ W1[e] + b1[e]) @ W2[e] + b2[e])
for x [8192, 1024], W1 [30, 1024, 2048], W2 [30, 2048].

Strategy: data-parallel over the batch axis. Each of the 8 cores gets a
1024-row shard of x (pre-transposed on host) and the full replicated
weight stack. No collectives: the host concatenates the 8 disjoint
output shards.

fc1 runs in fp8 (e4m3) with MatmulPerfMode.DoubleRow: two 128-wide
k-subtiles per matmul at 2 moving-rows/cycle, 2x the bf16 PE rate. W1
is scaled by 64 before the fp8 cast so its U(-1/32..) values sit in
e4m3's normal range; the 1/64 folds back in via the Relu eviction
scale. fc1 is 3840 N=512 DoubleRow matmuls at the ~216ns issue floor
(~830us of PE streaming per core) and is the roofline for this kernel.

fc2 runs OFF the PE (as replicated-stationary matmuls it would cost
~106us of PE time for 0.1% of the FLOPs). The Relu eviction writes h
as bf16; the otherwise-idle Vector engine computes
  s[p, b] += h[p, b] * W2[e, jb*128+p]
per column tile as a 4x-mode tensor_scalar multiply (per-partition
scalar) plus a 2x-mode tensor_tensor add. After 16 tiles, one matmul
per (expert, batch-half) with a [128, 1] ones stationary reduces s
over partitions into the logit (~216ns each, 60 total). Sigmoid reads
the 1-partition PSUM row, adds b2, and stages to the bf16 o_all tile;
the expert mean over experts 0..28 runs as a small matmul during
expert 29, and expert 29's contribution is folded in by a DVE
scalar_tensor_tensor (o29/E + mean_psum) so the post-loop tail is just
sigmoid -> DVE combine -> output DMA.

Scheduling: all boot-critical input DMA rides the Sync HWDGE queue in
just-in-time FIFO order (every logical DMA queue shares one ~213GB/s
pool of 16 SDMA engines, so ordering — not queue count — is what
matters; only the tiny expert-0/1 slices of b1/w2 load early, the bulk
follows the xt stream). The PE is warmed out of its cold p-state by
junk N=128 matmuls on a memset tile starting right after the fixed
~7us NEFF preamble. Experts 0 and 29 process all of batch-half 0
before batch-half 1: expert 0 so the PE stream is dense before xt's
second half has landed (a mid-ramp gap would re-arm the HAM throttle
window), expert 29 so batch-half 0's logit/sigmoid/output drains while
batch-half 1 is still in fc1, leaving a minimal tail. w1 is prefetched
8 pair-blocks deep (the split experts hold all 8 of their blocks
across both batch-half passes).

Layouts (prepared host-side in numpy, fp8 = ml_dtypes.float8_e4m3):
  xt  [BH, 128, IB, 512]      xt[t,p,s,n]    = x[t*512+n, s*128 + p]
  w1  [E*JP, 128, 2, IB, 128] w1[gp,p,u,s,q] = 64*W1[e, s*128+p, (2jp+u)*128+q]
  b1  [128, E*JB]             b1[p, e*JB+jb] = b1[e, jb*128 + p]
  w2  [128, E*JB]  (fp32)     w2[p, e*JB+jb] = W2[e, jb*128 + p]
  b2  [1, E]
"""

import numpy as np

import concourse.bacc as bacc
import concourse.mybir as mybir
import concourse.tile as tile
from concourse.bass_utils import run_bass_kernel_spmd

N_CORES = 8
P = 128
NB = 512  # matmul moving free dim (psum bank = 512 fp32)
SCALE = 64.0  # host-side premultiply of W1 before the fp8 cast

E_FULL, I_FULL, H_FULL, B_FULL = 30, 1024, 2048, 8192


def build_bass(E=E_FULL, I=I_FULL, H=H_FULL, BC=B_FULL // N_CORES):
    IB = I // P
    JB = H // P
    JP = JB // 2
    BH = BC // NB
    f32 = mybir.dt.float32
    f8 = mybir.dt.float8e4
    bf16 = mybir.dt.bfloat16
    DoubleRow = mybir.MatmulPerfMode.DoubleRow
    Relu = mybir.ActivationFunctionType.Relu
    Sigmoid = mybir.ActivationFunctionType.Sigmoid
    mult = mybir.AluOpType.mult
    add = mybir.AluOpType.add

    nc = bacc.Bacc(None, target_bir_lowering=False)
    xt_d = nc.dram_tensor("xt", [BC // NB, P, IB, NB], f8, kind="ExternalInput")
    w1_d = nc.dram_tensor("w1", [E * JP, P, 2, IB, P], f8, kind="ExternalInput")
    b1_d = nc.dram_tensor("b1", [P, E * JB], f32, kind="ExternalInput")
    w2_d = nc.dram_tensor("w2", [P, E * JB], f32, kind="ExternalInput")
    b2_d = nc.dram_tensor("b2", [1, E], f32, kind="ExternalInput")
    out_d = nc.dram_tensor("out", [1, BC], f32, kind="ExternalOutput")

    SPLIT = (0, E - 1)  # experts processed one batch-half at a time

    with tile.TileContext(nc) as tc:
        with (
            tc.tile_pool(name="const", bufs=1) as const_pool,
            tc.tile_pool(name="xt", bufs=1) as xt_pool,
            tc.tile_pool(name="w1", bufs=14) as w1_pool,
            tc.tile_pool(name="h", bufs=8) as h_pool,
            tc.tile_pool(name="t", bufs=5) as t_pool,
            tc.tile_pool(name="s", bufs=4) as s_pool,
            tc.tile_pool(name="osb", bufs=3) as osb_pool,
            tc.tile_pool(name="fc1ps", bufs=2, space="PSUM") as fc1_psum,
            tc.tile_pool(name="rps", bufs=2, space="PSUM") as r_psum,
            tc.tile_pool(name="mps", bufs=2, space="PSUM") as m_psum,
        ):
            # --- boot: all latency-critical DMA on the Sync HWDGE queue in
            # just-in-time FIFO order (every logical queue shares the same 16
            # SDMA engines, ~213GB/s total — ordering, not queue count, is
            # what matters); b1/w2/b2 trickle in on GpSimd's SWDGE queue ---
            w1_pre = {}

            def ensure_pre(gpi):
                if gpi < E * JP and gpi not in w1_pre:
                    w_t = w1_pool.tile(
                        [P, 2, IB, P], f8, tag="w1", name=f"w1p_{gpi}"
                    )
                    nc.sync.dma_start(w_t[:], w1_d[gpi])
                    w1_pre[gpi] = w_t

            xt_t = xt_pool.tile([P, BH, IB, NB], f8)
            b1_t = const_pool.tile([P, E * JB], f32)
            w2_t = const_pool.tile([P, E * JB], f32)
            b2_t = const_pool.tile([1, E], f32)
            ensure_pre(0)
            nc.sync.dma_start(xt_t[:, 0, 0:4, :], xt_d[0, :, 0:4, :])
            nc.sync.dma_start(xt_t[:, 0, 4:IB, :], xt_d[0, :, 4:IB, :])
            ensure_pre(1)
            # expert 0/1 bias + fc2-weight columns only (32KB) — the bulk
            # follows after the latency-critical stream
            nc.sync.dma_start(b1_t[:, 0:2 * JB], b1_d[:, 0:2 * JB])
            nc.sync.dma_start(w2_t[:, 0:2 * JB], w2_d[:, 0:2 * JB])
            nc.sync.dma_start(b2_t[:], b2_d[:])
            ensure_pre(2)
            ensure_pre(3)
            nc.sync.dma_start(xt_t[:, 1, 0:4, :], xt_d[1, :, 0:4, :])
            ensure_pre(4)
            nc.sync.dma_start(xt_t[:, 1, 4:IB, :], xt_d[1, :, 4:IB, :])
            for gpi in range(5, JP):
                ensure_pre(gpi)
            nc.sync.dma_start(b1_t[:, 2 * JB:], b1_d[:, 2 * JB:])
            nc.sync.dma_start(w2_t[:, 2 * JB:], w2_d[:, 2 * JB:])

            junk_t = const_pool.tile([P, 2 * P], bf16)
            nc.vector.memset(junk_t[:], 0.03)
            ones_t = const_pool.tile([P, 1], bf16)
            nc.vector.memset(ones_t[:], 1.0)
            dumm_t = const_pool.tile([1, 1], bf16)
            # load the sigmoid table set (contains Relu as filler) now, so no
            # ACT_TABLE_LOAD stalls the eviction stream mid-kernel
            nc.scalar.activation(dumm_t[:], junk_t[0:1, 0:1], Sigmoid)
            wm_ps = fc1_psum.tile([P, 2, NB], f32, tag="fc1", name="warm_ps")
            for wi in range(33):
                nc.tensor.matmul(
                    wm_ps[:, 0, 0:P],
                    junk_t[:, 0:P],
                    junk_t[:, P:2 * P],
                    start=True,
                    stop=True,
                )

            # expert-mean weights for experts 0..E-2; expert E-1 is folded in
            # by the final DVE combine
            cA_t = const_pool.tile([E - 1, 1], bf16)
            nc.any.memset(cA_t[:], 1.0 / E)
            o_all = const_pool.tile([E, BC], bf16)

            s_by_e = {}
            mean_ps = {}

            def dve_fold(e, jb, h_ap, s_ap, shape_tag):
                """s += h * w2[:, col] on the Vector engine (4x mul + 2x add)."""
                col = e * JB + jb
                w2col = w2_t[:, col:col + 1]
                if jb == 0:
                    nc.vector.tensor_scalar_mul(s_ap, h_ap, w2col)
                else:
                    t_t = t_pool.tile(
                        [P, 2, NB], bf16, tag="t", name=f"t_{col}_{shape_tag}"
                    )
                    t_ap = t_t[:, 0:h_ap.shape[1], :] if len(h_ap.shape) == 3 \
                        else t_t[:, 0, :]
                    nc.vector.tensor_scalar_mul(t_ap, h_ap, w2col)
                    nc.vector.tensor_add(s_ap, t_ap, s_ap)

            def fc1_step(e, jp):
                """fc1 for hidden pair (e, jp), both batch halves: 16
                DoubleRow matmuls + 2 Relu evictions + DVE w2-fold."""
                gp = e * JP + jp
                ensure_pre(gp + 8)
                if jp == 0:
                    s_by_e[e] = s_pool.tile(
                        [P, 2, NB], bf16, tag="s", name=f"s_{e}"
                    )
                w_t = w1_pre.pop(gp)
                for u in range(2):
                    jb = 2 * jp + u
                    col = e * JB + jb
                    ps = fc1_psum.tile(
                        [P, 2, NB], f32, tag="fc1", name=f"fc1ps_{col}"
                    )
                    for sb in range(0, IB, 2):
                        for bh in range(BH):
                            nc.tensor.matmul(
                                ps[:, bh, :],
                                w_t[:, u, sb:sb + 2, :],
                                xt_t[:, bh, sb:sb + 2, :],
                                start=(sb == 0),
                                stop=(sb == IB - 2),
                                perf_mode=DoubleRow,
                            )
                    h_t = h_pool.tile(
                        [P, 2, NB], bf16, tag="h", name=f"h_{col}"
                    )
                    nc.scalar.activation(
                        h_t[:],
                        ps[:],
                        Relu,
                        bias=b1_t[:, col:col + 1],
                        scale=1.0 / SCALE,
                    )
                    dve_fold(e, jb, h_t[:], s_by_e[e][:], "f")

            def split_group(e, jp, u, bh, w_t):
                """fc1 for hidden pair (e, jp), u, ONE batch half."""
                jb = 2 * jp + u
                col = e * JB + jb
                ps = fc1_psum.tile(
                    [P, 2, NB], f32, tag="fc1", name=f"fc1ps_{col}_{bh}"
                )
                for sb in range(0, IB, 2):
                    nc.tensor.matmul(
                        ps[:, bh, :],
                        w_t[:, u, sb:sb + 2, :],
                        xt_t[:, bh, sb:sb + 2, :],
                        start=(sb == 0),
                        stop=(sb == IB - 2),
                        perf_mode=DoubleRow,
                    )
                h_t = h_pool.tile(
                    [P, 2, NB], bf16, tag="h", name=f"h_{col}_{bh}"
                )
                nc.scalar.activation(
                    h_t[:, bh, :],
                    ps[:, bh, :],
                    Relu,
                    bias=b1_t[:, col:col + 1],
                    scale=1.0 / SCALE,
                )
                dve_fold(e, jb, h_t[:, bh, :], s_by_e[e][:, bh, :], f"s{bh}")

            def finish_expert(e):
                """partition-reduce s -> logit, sigmoid, stage to o_all."""
                s_t = s_by_e.pop(e)
                for bh in range(BH):
                    r_t = r_psum.tile([1, NB], f32, tag="r", name=f"r_{e}_{bh}")
                    nc.tensor.matmul(
                        r_t[:], ones_t[:], s_t[:, bh, :], start=True, stop=True
                    )
                    o_stage = osb_pool.tile(
                        [1, NB], bf16, tag="ostage", name=f"osig_{e}_{bh}"
                    )
                    nc.scalar.activation(
                        o_stage[:],
                        r_t[:],
                        Sigmoid,
                        bias=b2_t[0:1, e:e + 1],
                        scale=1.0,
                    )
                    nc.sync.dma_start(
                        o_all[e:e + 1, bh * NB:(bh + 1) * NB], o_stage[:]
                    )

            def mean_partial(bh):
                mps = m_psum.tile([1, NB], f32, tag="m", name=f"meanps_{bh}")
                nc.tensor.matmul(
                    mps[:],
                    cA_t[:],
                    o_all[0:E - 1, bh * NB:(bh + 1) * NB],
                    start=True,
                    stop=True,
                )
                mean_ps[bh] = mps

            def finish_last_chunk(bh, lo, hi):
                """expert E-1, batch [bh*NB+lo, bh*NB+hi): reduce, sigmoid,
                fold into the expert mean on DVE, write the output shard."""
                e = E - 1
                w = hi - lo
                r_t = r_psum.tile(
                    [1, NB], f32, tag="r", name=f"r_{e}_{bh}_{lo}"
                )
                nc.tensor.matmul(
                    r_t[0:1, 0:w],
                    ones_t[:],
                    s_by_e[e][:, bh, lo:hi],
                    start=True,
                    stop=True,
                )
                o_stage = osb_pool.tile(
                    [1, NB], bf16, tag="olast", name=f"osig_{e}_{bh}_{lo}"
                )
                nc.scalar.activation(
                    o_stage[0:1, 0:w], r_t[0:1, 0:w], Sigmoid,
                    bias=b2_t[0:1, e:e + 1], scale=1.0,
                )
                o_sb = osb_pool.tile(
                    [1, NB], f32, tag="ofin", name=f"of_{bh}_{lo}"
                )
                nc.vector.scalar_tensor_tensor(
                    o_sb[0:1, 0:w], o_stage[0:1, 0:w], 1.0 / E,
                    mean_ps[bh][0:1, lo:hi], mult, add
                )
                nc.scalar.dma_start(
                    out_d[0:1, bh * NB + lo:bh * NB + hi], o_sb[0:1, 0:w]
                )

            def tail_group(e, jp, u, bh, w_t):
                """like split_group, but in two 256-batch chunks with a
                single-op DVE fold and per-chunk finish, to minimize the
                post-loop dependency chain. Only used for the very last
                column of the last expert."""
                jb = 2 * jp + u
                col = e * JB + jb
                w2col = w2_t[:, col:col + 1]
                ps = fc1_psum.tile(
                    [P, 2, NB], f32, tag="fc1", name=f"fc1ps_{col}_{bh}"
                )
                h_t = h_pool.tile([P, 2, NB], bf16, tag="h", name=f"h_{col}_{bh}")
                HB = NB // 2
                for hf in range(2):
                    lo, hi = hf * HB, (hf + 1) * HB
                    for sb in range(0, IB, 2):
                        nc.tensor.matmul(
                            ps[:, bh, lo:hi],
                            w_t[:, u, sb:sb + 2, :],
                            xt_t[:, bh, sb:sb + 2, lo:hi],
                            start=(sb == 0),
                            stop=(sb == IB - 2),
                            perf_mode=DoubleRow,
                        )
                    nc.scalar.activation(
                        h_t[:, bh, lo:hi],
                        ps[:, bh, lo:hi],
                        Relu,
                        bias=b1_t[:, col:col + 1],
                        scale=1.0 / SCALE,
                    )
                    nc.vector.scalar_tensor_tensor(
                        s_by_e[e][:, bh, lo:hi],
                        h_t[:, bh, lo:hi],
                        w2col,
                        s_by_e[e][:, bh, lo:hi],
                        mult,
                        add,
                    )
                    finish_last_chunk(bh, lo, hi)

            def emit_split(e):
                w1_hold = {}
                for bh in range(BH):
                    for jp in range(JP):
                        gp = e * JP + jp
                        ensure_pre(gp + 8)
                        if bh == 0:
                            if jp == 0:
                                s_by_e[e] = s_pool.tile(
                                    [P, 2, NB], bf16, tag="s", name=f"s_{e}"
                                )
                            w1_hold[jp] = w1_pre.pop(gp)
                        w_t = w1_hold[jp]
                        for u in range(2):
                            if e == E - 1 and bh == 1 and jp == JP - 1 and u == 1:
                                tail_group(e, jp, u, bh, w_t)
                            else:
                                split_group(e, jp, u, bh, w_t)
                        if e == E - 1:
                            if bh == 0 and jp == 0:
                                finish_expert(e - 1)
                            if jp == 4:
                                mean_partial(bh)
                            if bh == 1 and jp == 0:
                                finish_last_chunk(0, 0, NB)
                if e == E - 1:
                    s_by_e.pop(e)
                w1_hold.clear()

            for e in range(E):
                if e in SPLIT:
                    emit_split(e)
                else:
                    for jp in range(JP):
                        fc1_step(e, jp)
                        if jp == 0 and e >= 1:
                            finish_expert(e - 1)
    nc.compile()
    return nc


def prep_inputs(x, W1, b1, W2, b2, E, I, H, BC):
    IB = I // P
    JB = H // P
    JP = JB // 2
    f8 = mybir.dt.np(mybir.dt.float8e4)
    w1_l = np.ascontiguousarray(
        (W1.astype(np.float32) * SCALE)
        .reshape(E, IB, P, JP, 2, P)
        .transpose(0, 3, 2, 4, 1, 5)
        .reshape(E * JP, P, 2, IB, P)
    ).astype(f8)
    b1_l = np.ascontiguousarray(
        b1.reshape(E, JB, P).transpose(2, 0, 1).reshape(P, E * JB), np.float32
    )
    w2_l = np.ascontiguousarray(
        W2.astype(np.float32).reshape(E, JB, P).transpose(2, 0, 1).reshape(P, E * JB)
    )
    b2_l = np.ascontiguousarray(b2.reshape(1, E), np.float32)
    in_maps = []
    for c in range(N_CORES):
        xc = np.asarray(x[c * BC:(c + 1) * BC], np.float32)  # [BC, I]
        xt = np.ascontiguousarray(
            xc.reshape(BC // 512, 512, IB, P).transpose(0, 3, 2, 1)
        ).astype(f8)
        in_maps.append({"xt": xt, "w1": w1_l, "b1": b1_l, "w2": w2_l, "b2": b2_l})
    return in_maps


def run(x, W1, b1, W2, b2, trace=False):
    E, I, H = W1.shape
    BC = x.shape[0] // N_CORES
    in_maps = prep_inputs(x, W1, b1, W2, b2, E, I, H, BC)
    nc = build_bass(E=E, I=I, H=H, BC=BC)
    res = run_bass_kernel_spmd(nc, in_maps, list(range(N_CORES)), trace=trace)
    outs = [res.results[c]["out"].reshape(BC) for c in range(N_CORES)]
    full = np.concatenate(outs)[:, None].astype(np.float32)
    return full, res


def kernel(x, W1, b1, W2, b2):
    out, _ = run(
        np.asarray(x), np.asarray(W1), np.asarray(b1), np.asarray(W2), np.asarray(b2)
    )
    return out
